# revision 21
# baseline (speedup 1.0000x reference)
"""Trainium2 Bass kernel for masked-CRF negative log-likelihood loss.

Problem: B=256, T=1024, N=128 tags. loss = mean_b( logZ_b - goldscore_b ).

Strategy (8 NeuronCores, data-parallel over batch, 32 batch rows per core):

  Forward algorithm in *exp space*: with expA = exp(tr[:N,:N]) held as the
  stationary PE weight, one recursion step is a 128x128 @ 128x32 matmul per
  chain followed by one elementwise multiply with exp(em_t - C) (DVE). A
  constant C ~= log(N) + 0.5 is folded into the emissions so state magnitude
  stays bounded; the deterministic offset t*C is added back at the end.

  The scan state is split into two independent 32-column chains (q partition
  chain / g gold chain) so the two per-step ops are narrower and PE/DVE
  round-trips overlap across chains.

  Emissions arrive from the host already transposed to [tag, t] (bf16) and
  the gold one-hots arrive as a host-built masked one-hot channel, so chunk
  production is two batched DMAs plus four Activation-engine exp ops per
  128-step chunk -- no PE transposes and no DVE one-hot generation. Gold
  emission scores accumulate on the otherwise-idle GpSimd engine with fused
  em*onehot row-reductions.

  Masking needs no selects: sequence lengths are >= T/2, so the scan runs
  unmasked and the state for t in [511, 1023] is written into an SBUF history
  buffer (bf16). Each pair of history slots is captured with one matmul
  against exp(tr[:, end]) in PE idle time; post-scan each batch selects its
  own end-step column and z = log(capture) + len*C. Per-core partial losses
  are summed on the host.
"""

import numpy as np

B, T, N = 256, 1024, 128
NCORES = 8
BC = B // NCORES            # 32 batch rows per core
CH = 128                    # scan steps per emission chunk
NJ = T // CH                # 8 chunks
TCAP0 = T // 2 - 1          # 511: first step whose state is kept in history
NCAP = T - TCAP0            # 513 history slots
SW = 2 * BC                 # 64 state columns: [q (partition) | g (gold)]
NCAP2 = NCAP + 1            # history padded to an even 514 slots
NBLK = NCAP2 // 2           # 257 two-slot capture blocks
CNORM = 5.354               # per-step log-scale removed from emissions

_CACHE = {}
LAST_EXEC_NS = None
TRACE = False
SKIP = set()
STAGE = 4


def _build(nc_mod):
    bass, mybir, tile, bacc = nc_mod
    f32 = mybir.dt.float32
    bf16 = mybir.dt.bfloat16
    u8 = mybir.dt.uint8
    AF = mybir.ActivationFunctionType
    OP = mybir.AluOpType

    nc = bacc.Bacc("TRN2", target_bir_lowering=False, debug=False,
                   num_devices=NCORES)

    emt_h = nc.dram_tensor("emt", [N, BC, T], bf16, kind="ExternalInput").ap()
    oht_h = nc.dram_tensor("oht", [N, BC, T], bf16, kind="ExternalInput").ap()
    mk_h = nc.dram_tensor("mk", [BC, T], u8, kind="ExternalInput").ap()
    tr_h = nc.dram_tensor("tr", [N + 2, N + 2], f32, kind="ExternalInput").ap()
    iota257_h = nc.dram_tensor("iota257", [128, NBLK], f32,
                               kind="ExternalInput").ap()
    parity_h = nc.dram_tensor("parity", [128, 1], f32, kind="ExternalInput").ap()
    sel2_h = nc.dram_tensor("sel2", [128, SW], f32, kind="ExternalInput").ap()
    loss_h = nc.dram_tensor("loss", [BC, 1], f32, kind="ExternalOutput").ap()

    with tile.TileContext(nc) as tc:
        with (
            tc.tile_pool(name="singles", bufs=1) as singles,
            tc.tile_pool(name="eo", bufs=2) as eo_pool,
            tc.tile_pool(name="emr", bufs=1) as emr_pool,
            tc.tile_pool(name="oht", bufs=2) as oht_pool,
            tc.tile_pool(name="scrg", bufs=2) as scrg_pool,
            tc.tile_pool(name="stA", bufs=3) as stA_pool,
            tc.tile_pool(name="stB", bufs=3) as stB_pool,
            tc.tile_pool(name="psA", bufs=2, space="PSUM") as psA_pool,
            tc.tile_pool(name="psB", bufs=2, space="PSUM") as psB_pool,
            tc.tile_pool(name="fpsum", bufs=1, space="PSUM") as fpsum_pool,
            tc.tile_pool(name="rpsum", bufs=2, space="PSUM") as rpsum_pool,
        ):
            # ---------------- constants / small inputs ----------------
            iota257_sb = singles.tile([128, NBLK], f32, tag="iota257")
            nc.sync.dma_start(iota257_sb[:], iota257_h[:, :])
            parity_sb = singles.tile([128, 1], f32, tag="parity")
            nc.sync.dma_start(parity_sb[:], parity_h[:, :])
            sel2_sb = singles.tile([128, SW], f32, tag="sel2")
            nc.sync.dma_start(sel2_sb[:], sel2_h[:, :])

            tr_sb = singles.tile([128, N + 2], f32, tag="tr")
            nc.sync.dma_start(tr_sb[:], tr_h[0:128, :])
            trs_sb = singles.tile([128, 1], f32, tag="trs")
            nc.sync.dma_start(trs_sb[:],
                              tr_h[128:129, 0:128].rearrange("r j -> j r"))

            ones128 = singles.tile([128, 1], f32, tag="ones")
            nc.vector.memset(ones128[:], 1.0)
            zero_b = singles.tile([128, 1], f32, tag="zero_b")
            nc.vector.memset(zero_b[:], 0.0)
            negc_b = singles.tile([128, 1], f32, tag="negc_b")
            nc.vector.memset(negc_b[:], -CNORM)

            expA = singles.tile([128, 128], bf16, tag="expA")
            nc.scalar.activation(expA[:], tr_sb[:, 0:128], AF.Exp, bias=zero_b[:])
            expEnd = singles.tile([128, 1], bf16, tag="expEnd")
            nc.scalar.activation(expEnd[:], tr_sb[:, 129:130], AF.Exp,
                                 bias=zero_b[:])
            expTrS = singles.tile([128, 1], f32, tag="expTrS")
            nc.scalar.activation(expTrS[:], trs_sb[:], AF.Exp, bias=zero_b[:])

            # mask transposed to [t-partition, (b,k)-free] column form.
            mkT = singles.tile([128, BC * NJ], u8, tag="mkT")
            nc.sync.dma_start(mkT[:],
                              mk_h.rearrange("b (k t) -> t (b k)", t=128))
            mkT_f = singles.tile([128, BC * NJ], f32, tag="mkT_f")
            nc.vector.tensor_copy(mkT_f[:], mkT[:])

            # sequence lengths: le[b] = sum_t mk[b, t]
            le_pb = singles.tile([128, BC], f32, tag="le_pb")
            nc.vector.tensor_reduce(
                le_pb[:], mkT_f[:].rearrange("p (b k) -> p b k", k=NJ),
                axis=mybir.AxisListType.X, op=OP.add)
            le_ps = fpsum_pool.tile([BC, 1], f32, tag="fin")
            nc.tensor.matmul(le_ps[:], le_pb[:], ones128[:], start=True, stop=True)
            le_s = singles.tile([BC, 1], f32, tag="le_s")
            nc.scalar.copy(le_s[:], le_ps[:])

            # le replicated to all four 32-partition groups: le4[p] = le[p % 32]
            le4 = singles.tile([128, 1], f32, tag="le4")
            for g in range(4):
                nc.sync.dma_start(le4[32 * g:32 * (g + 1), :], le_s[:])

            # capture block index per partition p=(par, c):
            #   slot* = le - 512; kidx = slot*/2 if slot* % 2 == par else -1
            kidx = singles.tile([128, 1], f32, tag="kidx")
            nc.vector.tensor_scalar(kidx[:], le4[:], -float(TCAP0 + 1),
                                    parity_sb[:], OP.add, OP.subtract)
            kidx_i = singles.tile([128, 1], mybir.dt.int32, tag="kidx_i")
            nc.vector.tensor_copy(kidx_i[:], kidx[:])
            odd_i = singles.tile([128, 1], mybir.dt.int32, tag="odd_i")
            nc.vector.tensor_scalar(odd_i[:], kidx_i[:], 1, None, OP.bitwise_and)
            nc.vector.tensor_scalar(kidx_i[:], kidx_i[:], 1, None,
                                    OP.arith_shift_right)
            odd_f = singles.tile([128, 1], f32, tag="odd_f")
            nc.vector.tensor_copy(odd_f[:], odd_i[:])
            nc.vector.tensor_copy(kidx[:], kidx_i[:])
            # kidx = kidx*(1-odd) - odd
            nevenf = singles.tile([128, 1], f32, tag="nevenf")
            nc.vector.tensor_scalar(nevenf[:], odd_f[:], -1.0, 1.0, OP.mult, OP.add)
            nc.vector.tensor_tensor(kidx[:], kidx[:], nevenf[:], OP.mult)
            nc.vector.tensor_tensor(kidx[:], kidx[:], odd_f[:], OP.subtract)

            # gold emission accumulator, one column per (b, group) tile
            emgold_cols = singles.tile([128, BC * 2], f32, tag="emgold")

            # state history for t in [TCAP0, T), padded to an even slot count
            qhist = singles.tile([128, NCAP2 * SW], bf16, tag="qhist")
            nc.vector.memset(qhist[:, NCAP * SW:], 0.0)
            # per-2-slot capture results: rall[p=(par,c), blk]
            rall_sb = singles.tile([128, NBLK], f32, tag="rall")

            # ---------------- emission/one-hot production ------------
            # 512-step group buffers (b-major): emr4/oh4 [128, (b t4)] bf16.
            # eob holds exp(em - C) for one 128-step chunk, b-major.
            GR = 4 * CH

            def produce_group(j4):
                emr4 = emr_pool.tile([128, BC * GR], bf16, tag="emr4")
                oh4 = oht_pool.tile([128, BC * GR], bf16, tag="oh4")
                nc.sync.dma_start(
                    emr4[:].rearrange("p (b t) -> p b t", t=GR),
                    emt_h[:, :, j4 * GR:(j4 + 1) * GR])
                nc.sync.dma_start(
                    oh4[:].rearrange("p (b t) -> p b t", t=GR),
                    oht_h[:, :, j4 * GR:(j4 + 1) * GR])
                return emr4, oh4

            def produce_chunk(c, emr4):
                # E = exp(em - C) for local chunk c of a group
                eob = eo_pool.tile([128, BC * CH], bf16, tag="eob")
                for g8 in range(4):
                    src = emr4[:].rearrange("p (b t) -> p b t", t=GR)[
                        :, 8 * g8:8 * (g8 + 1), c * CH:(c + 1) * CH]
                    dst = eob[:, g8 * 8 * CH:(g8 + 1) * 8 * CH].rearrange(
                        "p (b t) -> p b t", t=CH)
                    nc.scalar.activation(dst, src, AF.Exp, bias=negc_b[:])
                return eob

            def emgold_op(j4, b, emr4, oh4):
                # gold emission partial sums: sum_t em[tag,t]*onehot[tag,t]
                k = b * 2 + j4
                scr = scrg_pool.tile([128, GR], bf16, tag="scr")
                nc.vector.scalar_tensor_tensor(
                    scr[:], emr4[:, b * GR:(b + 1) * GR], 1.0,
                    oh4[:, b * GR:(b + 1) * GR],
                    OP.mult, OP.mult, accum_out=emgold_cols[:, k:k + 1])

            def eo_step_ap(eob, th):
                return eob[:].rearrange("p (b t) -> p t b", t=CH)[:, th:th + 1, :]

            def oh_step_ap(oh4, t4):
                return oh4[:].rearrange("p (b t) -> p t b", t=GR)[:, t4:t4 + 1, :]

            # group 0 + chunk 0 produced up front
            emr4_cur, oh4_cur = produce_group(0)
            eob_cur = produce_chunk(0, emr4_cur)
            emr4_nxt = oh4_nxt = None

            # ---------------- scan init (t = 0) ----------------------
            stA = stA_pool.tile([128, BC], bf16, tag="stA")
            stB = stB_pool.tile([128, BC], bf16, tag="stB")
            nc.vector.tensor_scalar(stA[:], eo_step_ap(eob_cur, 0),
                                    expTrS[:], None, OP.mult)
            nc.vector.tensor_scalar(stB[:], oh_step_ap(oh4_cur, 0),
                                    expTrS[:], None, OP.mult)
            sA = stA[:]
            sB = stB[:]

            # ---------------- the scan ------------------------------
            def capture_block(i):
                # r[par*64+c, i] = sum_j qhist[j, (2i+par, c)] * expEnd[j]
                nonlocal rall_ps
                if i % 128 == 0:
                    rall_ps = rpsum_pool.tile([128, 128], f32, tag="rps")
                nc.tensor.matmul(rall_ps[:, i % 128:i % 128 + 1],
                                 qhist[:, 2 * i * SW:(2 * i + 2) * SW],
                                 expEnd[:], start=True, stop=True)
                if i % 128 == 127 or i == NBLK - 1:
                    lo = (i // 128) * 128
                    nc.scalar.copy(rall_sb[:, lo:i + 1],
                                   rall_ps[:, 0:i + 1 - lo])

            rall_ps = None
            eob_next = None
            for t in range(1, T):
                j, th = divmod(t, CH)
                if th == 0:
                    if j % 4 == 0:
                        emr4_cur, oh4_cur = emr4_nxt, oh4_nxt
                    eob_cur = eob_next
                if j < NJ - 1 and th == 1:
                    nj = j + 1
                    eob_next = produce_chunk(
                        nj % 4, emr4_cur if nj % 4 != 0 else emr4_nxt)
                # group 1 DMA after group 0's em-gold ops have consumed emr4
                if t == 360:
                    emr4_nxt, oh4_nxt = produce_group(1)
                # one big em-gold op per batch, spread across the group
                r = t - (j // 4) * GR
                if 128 <= r < 128 + 7 * BC and (r - 128) % 7 == 0:
                    emgold_op(j // 4, (r - 128) // 7, emr4_cur, oh4_cur)

                psA = psA_pool.tile([128, BC], f32, tag="psA")
                nc.tensor.matmul(psA[:], expA[:], sA, start=True, stop=True)
                psB = psB_pool.tile([128, BC], f32, tag="psB")
                nc.tensor.matmul(psB[:], expA[:], sB, start=True, stop=True)
                aptA = eo_step_ap(eob_cur, th)
                aptB = oh_step_ap(oh4_cur, (j % 4) * CH + th)
                if t >= TCAP0:
                    slot = t - TCAP0
                    nxtA = qhist[:, slot * SW:slot * SW + BC]
                    nxtB = qhist[:, slot * SW + BC:(slot + 1) * SW]
                else:
                    tA = stA_pool.tile([128, BC], bf16, tag="stA")
                    tB = stB_pool.tile([128, BC], bf16, tag="stB")
                    nxtA = tA[:]
                    nxtB = tB[:]
                nc.vector.tensor_tensor(nxtA, psA[:], aptA, OP.mult)
                nc.vector.tensor_tensor(nxtB, psB[:], aptB, OP.mult)
                sA, sB = nxtA, nxtB
                # two fresh history slots -> one capture matmul (PE idle time)
                if t > TCAP0 and (t - TCAP0) % 2 == 1:
                    capture_block((t - TCAP0) // 2)
            capture_block(NBLK - 1)

            # ------- select each partition's own capture block -------
            selm = singles.tile([128, NBLK], f32, tag="selm")
            nc.vector.tensor_scalar(selm[:], iota257_sb[:], kidx[:], None,
                                    OP.is_equal)
            scr2 = singles.tile([128, NBLK], f32, tag="scr2")
            rsel = singles.tile([128, 1], f32, tag="rsel")
            nc.vector.tensor_tensor(scr2[:], rall_sb[:], selm[:], OP.mult)
            nc.vector.tensor_reduce(rsel[:], scr2[:], axis=mybir.AxisListType.X,
                                    op=OP.add)
            # collapse the two parity partitions: rstar64[c] = sum_par rsel
            rstar_ps = fpsum_pool.tile([SW, 1], f32, tag="fin")
            nc.tensor.matmul(rstar_ps[:], sel2_sb[:], rsel[:],
                             start=True, stop=True)
            ln64 = singles.tile([SW, 1], f32, tag="ln64")
            nc.scalar.activation(ln64[:], rstar_ps[:], AF.Ln,
                                 bias=zero_b[0:SW, :])
            lng32 = singles.tile([BC, 1], f32, tag="lng32")
            nc.sync.dma_start(lng32[:], ln64[BC:SW, :])

            emgold_pb = singles.tile([128, BC], f32, tag="emgold_pb")
            nc.vector.tensor_reduce(
                emgold_pb[:], emgold_cols[:].rearrange("p (b k) -> p b k", k=2),
                axis=mybir.AxisListType.X, op=OP.add)
            emg_ps = fpsum_pool.tile([BC, 1], f32, tag="fin")
            nc.tensor.matmul(emg_ps[:], emgold_pb[:], ones128[:],
                             start=True, stop=True)

            # ---------------- final loss assembly --------------------
            emg_s = singles.tile([BC, 1], f32, tag="emg_s")
            nc.scalar.copy(emg_s[:], emg_ps[:])
            lec = singles.tile([BC, 1], f32, tag="lec")
            nc.vector.tensor_scalar(lec[:], le_s[:], CNORM, None, OP.mult)

            loss_s = singles.tile([BC, 1], f32, tag="loss_s")
            nc.vector.tensor_tensor(loss_s[:], ln64[0:BC, :], lng32[:],
                                    OP.subtract)
            nc.vector.tensor_tensor(loss_s[:], loss_s[:], emg_s[:], OP.subtract)
            nc.vector.tensor_tensor(loss_s[:], loss_s[:], lec[:], OP.add)
            nc.sync.dma_start(loss_h[:, :], loss_s[:])

    nc.compile()
    return nc


def _get_nc():
    key = (tuple(sorted(SKIP)), STAGE)
    if key not in _CACHE:
        import concourse.bass as bass
        import concourse.mybir as mybir
        import concourse.tile as tile
        import concourse.bacc as bacc
        _CACHE[key] = _build((bass, mybir, tile, bacc))
    return _CACHE[key]


def kernel(em, tg, mk, tr):
    global LAST_EXEC_NS
    import ml_dtypes
    from concourse import bass_utils

    nc = _get_nc()
    bf16 = ml_dtypes.bfloat16

    iota257 = np.broadcast_to(np.arange(NBLK, dtype=np.float32),
                              (128, NBLK)).copy()
    parity = (np.arange(128, dtype=np.float32) // 64).reshape(128, 1).copy()
    sel2 = (np.arange(128).reshape(128, 1) % 64 ==
            np.arange(SW).reshape(1, SW)).astype(np.float32)

    em = np.asarray(em, dtype=np.float32)
    tg = np.asarray(tg, dtype=np.int64)
    mkb = np.asarray(mk, dtype=bool)
    ar = np.arange(N, dtype=np.int64)[None, :, None]

    in_maps = []
    for c in range(NCORES):
        sl = slice(c * BC, (c + 1) * BC)
        emt = np.ascontiguousarray(
            em[sl].transpose(2, 0, 1)).astype(bf16)
        oht = ((tg[sl][None, :, :] == ar.transpose(1, 0, 2)) &
               mkb[sl][None, :, :]).astype(bf16)
        in_maps.append({
            "emt": emt,
            "oht": np.ascontiguousarray(oht),
            "mk": np.ascontiguousarray(mkb[sl], dtype=np.uint8),
            "tr": np.ascontiguousarray(tr, dtype=np.float32),
            "iota257": iota257,
            "parity": parity,
            "sel2": sel2,
        })

    import os
    res = bass_utils.run_bass_kernel_spmd(
        nc, in_maps, core_ids=list(range(NCORES)), trace=TRACE,
        tmpdir=os.environ.get("BASS_TMPDIR") or None)
    LAST_EXEC_NS = res.exec_time_ns

    parts = [res.results[c]["loss"].reshape(-1) for c in range(NCORES)]
    total = np.concatenate(parts).astype(np.float64)
    return np.float32(total.mean())


# revision 22
# speedup vs baseline: 1.0721x; 1.0721x over previous
"""Trainium2 Bass kernel for masked-CRF negative log-likelihood loss.

Problem: B=256, T=1024, N=128 tags. loss = mean_b( logZ_b - goldscore_b ).

Strategy (8 NeuronCores, data-parallel over batch, 32 batch rows per core):

  Forward algorithm in *exp space*: with expA = exp(tr[:N,:N]) held as the
  stationary PE weight, one recursion step is a 128x128 @ 128x32 matmul per
  chain followed by one elementwise multiply with exp(em_t - C) (DVE). A
  constant C ~= log(N) + 0.5 is folded into the emissions so state magnitude
  stays bounded; the deterministic offset t*C is added back at the end.

  The scan state is split into two independent 32-column chains (q partition
  chain / g gold chain) so the two per-step ops are narrower and PE/DVE
  round-trips overlap across chains.

  Emissions arrive from the host already transposed to [tag, t] (bf16) and
  the gold one-hots arrive as a host-built masked one-hot channel, so chunk
  production is two batched DMAs plus four Activation-engine exp ops per
  128-step chunk -- no PE transposes and no DVE one-hot generation. Gold
  emission scores accumulate on the otherwise-idle GpSimd engine with fused
  em*onehot row-reductions.

  Masking needs no selects: sequence lengths are >= T/2, so the scan runs
  unmasked and the state for t in [511, 1023] is written into an SBUF history
  buffer (bf16). Each pair of history slots is captured with one matmul
  against exp(tr[:, end]) in PE idle time; post-scan each batch selects its
  own end-step column and z = log(capture) + len*C. Per-core partial losses
  are summed on the host.
"""

import numpy as np

B, T, N = 256, 1024, 128
NCORES = 8
BC = B // NCORES            # 32 batch rows per core
CH = 128                    # scan steps per emission chunk
NJ = T // CH                # 8 chunks
TCAP0 = T // 2 - 1          # 511: first step whose state is kept in history
NCAP = T - TCAP0            # 513 history slots
SW = 2 * BC                 # 64 state columns: [q (partition) | g (gold)]
NCAP2 = NCAP + 1            # history padded to an even 514 slots
NBLK = NCAP2 // 2           # 257 two-slot capture blocks
CNORM = 5.354               # per-step log-scale removed from emissions

_CACHE = {}
LAST_EXEC_NS = None
TRACE = False
SKIP = set()
STAGE = 4


def _build(nc_mod):
    bass, mybir, tile, bacc = nc_mod
    f32 = mybir.dt.float32
    bf16 = mybir.dt.bfloat16
    u8 = mybir.dt.uint8
    AF = mybir.ActivationFunctionType
    OP = mybir.AluOpType

    nc = bacc.Bacc("TRN2", target_bir_lowering=False, debug=False,
                   num_devices=NCORES)

    emt_h = nc.dram_tensor("emt", [N, BC, T], bf16, kind="ExternalInput").ap()
    oht_h = nc.dram_tensor("oht", [N, BC, T], bf16, kind="ExternalInput").ap()
    mk_h = nc.dram_tensor("mk", [BC, T], u8, kind="ExternalInput").ap()
    tr_h = nc.dram_tensor("tr", [N + 2, N + 2], f32, kind="ExternalInput").ap()
    iota257_h = nc.dram_tensor("iota257", [128, NBLK], f32,
                               kind="ExternalInput").ap()
    parity_h = nc.dram_tensor("parity", [128, 1], f32, kind="ExternalInput").ap()
    sel2_h = nc.dram_tensor("sel2", [128, SW], f32, kind="ExternalInput").ap()
    loss_h = nc.dram_tensor("loss", [BC, 1], f32, kind="ExternalOutput").ap()

    with tile.TileContext(nc) as tc:
        with (
            tc.tile_pool(name="singles", bufs=1) as singles,
            tc.tile_pool(name="eo", bufs=2) as eo_pool,
            tc.tile_pool(name="emr", bufs=2) as emr_pool,
            tc.tile_pool(name="scrg", bufs=2) as scrg_pool,
            tc.tile_pool(name="stA", bufs=3) as stA_pool,
            tc.tile_pool(name="stB", bufs=3) as stB_pool,
            tc.tile_pool(name="psA", bufs=2, space="PSUM") as psA_pool,
            tc.tile_pool(name="psB", bufs=2, space="PSUM") as psB_pool,
            tc.tile_pool(name="fpsum", bufs=1, space="PSUM") as fpsum_pool,
            tc.tile_pool(name="rpsum", bufs=2, space="PSUM") as rpsum_pool,
        ):
            # ---------------- constants / small inputs ----------------
            iota257_sb = singles.tile([128, NBLK], f32, tag="iota257")
            nc.sync.dma_start(iota257_sb[:], iota257_h[:, :])
            parity_sb = singles.tile([128, 1], f32, tag="parity")
            nc.sync.dma_start(parity_sb[:], parity_h[:, :])
            sel2_sb = singles.tile([128, SW], f32, tag="sel2")
            nc.sync.dma_start(sel2_sb[:], sel2_h[:, :])

            tr_sb = singles.tile([128, N + 2], f32, tag="tr")
            nc.sync.dma_start(tr_sb[:], tr_h[0:128, :])
            trs_sb = singles.tile([128, 1], f32, tag="trs")
            nc.sync.dma_start(trs_sb[:],
                              tr_h[128:129, 0:128].rearrange("r j -> j r"))

            ones128 = singles.tile([128, 1], f32, tag="ones")
            nc.vector.memset(ones128[:], 1.0)
            zero_b = singles.tile([128, 1], f32, tag="zero_b")
            nc.vector.memset(zero_b[:], 0.0)
            negc_b = singles.tile([128, 1], f32, tag="negc_b")
            nc.vector.memset(negc_b[:], -CNORM)

            expA = singles.tile([128, 128], bf16, tag="expA")
            nc.scalar.activation(expA[:], tr_sb[:, 0:128], AF.Exp, bias=zero_b[:])
            expEnd = singles.tile([128, 1], bf16, tag="expEnd")
            nc.scalar.activation(expEnd[:], tr_sb[:, 129:130], AF.Exp,
                                 bias=zero_b[:])
            expTrS = singles.tile([128, 1], f32, tag="expTrS")
            nc.scalar.activation(expTrS[:], trs_sb[:], AF.Exp, bias=zero_b[:])

            # mask transposed to [t-partition, (b,k)-free] column form.
            mkT = singles.tile([128, BC * NJ], u8, tag="mkT")
            nc.sync.dma_start(mkT[:],
                              mk_h.rearrange("b (k t) -> t (b k)", t=128))
            mkT_f = singles.tile([128, BC * NJ], f32, tag="mkT_f")
            nc.vector.tensor_copy(mkT_f[:], mkT[:])

            # sequence lengths: le[b] = sum_t mk[b, t]
            le_pb = singles.tile([128, BC], f32, tag="le_pb")
            nc.vector.tensor_reduce(
                le_pb[:], mkT_f[:].rearrange("p (b k) -> p b k", k=NJ),
                axis=mybir.AxisListType.X, op=OP.add)
            le_ps = fpsum_pool.tile([BC, 1], f32, tag="fin")
            nc.tensor.matmul(le_ps[:], le_pb[:], ones128[:], start=True, stop=True)
            le_s = singles.tile([BC, 1], f32, tag="le_s")
            nc.scalar.copy(le_s[:], le_ps[:])

            # le replicated to all four 32-partition groups: le4[p] = le[p % 32]
            le4 = singles.tile([128, 1], f32, tag="le4")
            for g in range(4):
                nc.sync.dma_start(le4[32 * g:32 * (g + 1), :], le_s[:])

            # capture block index per partition p=(par, c):
            #   slot* = le - 512; kidx = slot*/2 if slot* % 2 == par else -1
            kidx = singles.tile([128, 1], f32, tag="kidx")
            nc.vector.tensor_scalar(kidx[:], le4[:], -float(TCAP0 + 1),
                                    parity_sb[:], OP.add, OP.subtract)
            kidx_i = singles.tile([128, 1], mybir.dt.int32, tag="kidx_i")
            nc.vector.tensor_copy(kidx_i[:], kidx[:])
            odd_i = singles.tile([128, 1], mybir.dt.int32, tag="odd_i")
            nc.vector.tensor_scalar(odd_i[:], kidx_i[:], 1, None, OP.bitwise_and)
            nc.vector.tensor_scalar(kidx_i[:], kidx_i[:], 1, None,
                                    OP.arith_shift_right)
            odd_f = singles.tile([128, 1], f32, tag="odd_f")
            nc.vector.tensor_copy(odd_f[:], odd_i[:])
            nc.vector.tensor_copy(kidx[:], kidx_i[:])
            # kidx = kidx*(1-odd) - odd
            nevenf = singles.tile([128, 1], f32, tag="nevenf")
            nc.vector.tensor_scalar(nevenf[:], odd_f[:], -1.0, 1.0, OP.mult, OP.add)
            nc.vector.tensor_tensor(kidx[:], kidx[:], nevenf[:], OP.mult)
            nc.vector.tensor_tensor(kidx[:], kidx[:], odd_f[:], OP.subtract)

            # gold emission accumulator, one column per (b, j) tile
            emgold_cols = singles.tile([128, BC * NJ], f32, tag="emgold")

            # state history for t in [TCAP0, T), padded to an even slot count
            qhist = singles.tile([128, NCAP2 * SW], bf16, tag="qhist")
            nc.vector.memset(qhist[:, NCAP * SW:], 0.0)
            # per-2-slot capture results: rall[p=(par,c), blk]
            rall_sb = singles.tile([128, NBLK], f32, tag="rall")

            # ---------------- emission/one-hot chunk production ------
            # eob layout: [128, (g t)] bf16 with g in [0,64): g<32 -> E col of
            # batch g, g>=32 -> one-hot col of batch g-32 (both b-major).
            def produce_chunk(j):
                eob = eo_pool.tile([128, 2 * BC * CH], bf16, tag="eob")
                emr = emr_pool.tile([128, BC * CH], bf16, tag="emr")
                nc.sync.dma_start(
                    emr[:].rearrange("p (b t) -> p b t", t=CH),
                    emt_h[:, :, j * CH:(j + 1) * CH])
                nc.sync.dma_start(
                    eob[:, BC * CH:2 * BC * CH].rearrange(
                        "p (b t) -> p b t", t=CH),
                    oht_h[:, :, j * CH:(j + 1) * CH])
                # E = exp(em - C), 8 batches per Activation op
                for g4 in range(4):
                    nc.scalar.activation(
                        eob[:, g4 * 8 * CH:(g4 + 1) * 8 * CH],
                        emr[:, g4 * 8 * CH:(g4 + 1) * 8 * CH],
                        AF.Exp, bias=negc_b[:])
                return eob, emr

            def emgold_op(j, b, eob, emr):
                # gold emission partial sums: sum_t em[tag,t]*onehot[tag,t]
                # (SBUF-only, so it runs on the otherwise-idle GpSimd engine)
                k = b * NJ + j
                scr = scrg_pool.tile([128, CH], bf16, tag="scr")
                nc.vector.scalar_tensor_tensor(
                    scr[:], emr[:, b * CH:(b + 1) * CH], 1.0,
                    eob[:, BC * CH + b * CH:BC * CH + (b + 1) * CH],
                    OP.mult, OP.mult, accum_out=emgold_cols[:, k:k + 1])

            def eo_step_ap(eob, th):
                # [E_t cols (32) | OH_t cols (32)] as one strided [128,1,64] AP
                return eob[:].rearrange("p (g t) -> p t g", t=CH)[:, th:th + 1, :]

            # chunk 0 produced up front
            eob_cur, emr_cur = produce_chunk(0)

            # ---------------- scan init (t = 0) ----------------------
            stA = stA_pool.tile([128, BC], bf16, tag="stA")
            stB = stB_pool.tile([128, BC], bf16, tag="stB")
            ap0 = eo_step_ap(eob_cur, 0)
            nc.vector.tensor_scalar(stA[:], ap0[:, :, 0:BC],
                                    expTrS[:], None, OP.mult)
            nc.vector.tensor_scalar(stB[:], ap0[:, :, BC:SW],
                                    expTrS[:], None, OP.mult)
            sA = stA[:]
            sB = stB[:]

            # ---------------- the scan ------------------------------
            def capture_block(i):
                # r[par*64+c, i] = sum_j qhist[j, (2i+par, c)] * expEnd[j]
                nonlocal rall_ps
                if i % 128 == 0:
                    rall_ps = rpsum_pool.tile([128, 128], f32, tag="rps")
                nc.tensor.matmul(rall_ps[:, i % 128:i % 128 + 1],
                                 qhist[:, 2 * i * SW:(2 * i + 2) * SW],
                                 expEnd[:], start=True, stop=True)
                if i % 128 == 127 or i == NBLK - 1:
                    lo = (i // 128) * 128
                    nc.scalar.copy(rall_sb[:, lo:i + 1],
                                   rall_ps[:, 0:i + 1 - lo])

            rall_ps = None
            eob_next = emr_next = None
            for t in range(1, T):
                j, th = divmod(t, CH)
                if th == 0:
                    eob_cur, emr_cur = eob_next, emr_next
                if j < NJ - 1 and th == 1:
                    eob_next, emr_next = produce_chunk(j + 1)
                if 64 <= th < 64 + BC:
                    emgold_op(j, th - 64, eob_cur, emr_cur)

                psA = psA_pool.tile([128, BC], f32, tag="psA")
                nc.tensor.matmul(psA[:], expA[:], sA, start=True, stop=True)
                psB = psB_pool.tile([128, BC], f32, tag="psB")
                nc.tensor.matmul(psB[:], expA[:], sB, start=True, stop=True)
                apt = eo_step_ap(eob_cur, th)
                if t >= TCAP0:
                    slot = t - TCAP0
                    nxtA = qhist[:, slot * SW:slot * SW + BC]
                    nxtB = qhist[:, slot * SW + BC:(slot + 1) * SW]
                else:
                    tA = stA_pool.tile([128, BC], bf16, tag="stA")
                    tB = stB_pool.tile([128, BC], bf16, tag="stB")
                    nxtA = tA[:]
                    nxtB = tB[:]
                nc.vector.tensor_tensor(nxtA, psA[:], apt[:, :, 0:BC], OP.mult)
                nc.vector.tensor_tensor(nxtB, psB[:], apt[:, :, BC:SW], OP.mult)
                sA, sB = nxtA, nxtB
                # two fresh history slots -> one capture matmul (PE idle time)
                if t > TCAP0 and (t - TCAP0) % 2 == 1:
                    capture_block((t - TCAP0) // 2)
            capture_block(NBLK - 1)

            # ------- select each partition's own capture block -------
            selm = singles.tile([128, NBLK], f32, tag="selm")
            nc.vector.tensor_scalar(selm[:], iota257_sb[:], kidx[:], None,
                                    OP.is_equal)
            scr2 = singles.tile([128, NBLK], f32, tag="scr2")
            rsel = singles.tile([128, 1], f32, tag="rsel")
            nc.vector.tensor_tensor(scr2[:], rall_sb[:], selm[:], OP.mult)
            nc.vector.tensor_reduce(rsel[:], scr2[:], axis=mybir.AxisListType.X,
                                    op=OP.add)
            # collapse the two parity partitions: rstar64[c] = sum_par rsel
            rstar_ps = fpsum_pool.tile([SW, 1], f32, tag="fin")
            nc.tensor.matmul(rstar_ps[:], sel2_sb[:], rsel[:],
                             start=True, stop=True)
            ln64 = singles.tile([SW, 1], f32, tag="ln64")
            nc.scalar.activation(ln64[:], rstar_ps[:], AF.Ln,
                                 bias=zero_b[0:SW, :])
            lng32 = singles.tile([BC, 1], f32, tag="lng32")
            nc.sync.dma_start(lng32[:], ln64[BC:SW, :])

            emgold_pb = singles.tile([128, BC], f32, tag="emgold_pb")
            nc.vector.tensor_reduce(
                emgold_pb[:], emgold_cols[:].rearrange("p (b k) -> p b k", k=NJ),
                axis=mybir.AxisListType.X, op=OP.add)
            emg_ps = fpsum_pool.tile([BC, 1], f32, tag="fin")
            nc.tensor.matmul(emg_ps[:], emgold_pb[:], ones128[:],
                             start=True, stop=True)

            # ---------------- final loss assembly --------------------
            emg_s = singles.tile([BC, 1], f32, tag="emg_s")
            nc.scalar.copy(emg_s[:], emg_ps[:])
            lec = singles.tile([BC, 1], f32, tag="lec")
            nc.vector.tensor_scalar(lec[:], le_s[:], CNORM, None, OP.mult)

            loss_s = singles.tile([BC, 1], f32, tag="loss_s")
            nc.vector.tensor_tensor(loss_s[:], ln64[0:BC, :], lng32[:],
                                    OP.subtract)
            nc.vector.tensor_tensor(loss_s[:], loss_s[:], emg_s[:], OP.subtract)
            nc.vector.tensor_tensor(loss_s[:], loss_s[:], lec[:], OP.add)
            nc.sync.dma_start(loss_h[:, :], loss_s[:])

    nc.compile()
    return nc


def _get_nc():
    key = (tuple(sorted(SKIP)), STAGE)
    if key not in _CACHE:
        import concourse.bass as bass
        import concourse.mybir as mybir
        import concourse.tile as tile
        import concourse.bacc as bacc
        _CACHE[key] = _build((bass, mybir, tile, bacc))
    return _CACHE[key]


def kernel(em, tg, mk, tr):
    global LAST_EXEC_NS
    import ml_dtypes
    from concourse import bass_utils

    nc = _get_nc()
    bf16 = ml_dtypes.bfloat16

    iota257 = np.broadcast_to(np.arange(NBLK, dtype=np.float32),
                              (128, NBLK)).copy()
    parity = (np.arange(128, dtype=np.float32) // 64).reshape(128, 1).copy()
    sel2 = (np.arange(128).reshape(128, 1) % 64 ==
            np.arange(SW).reshape(1, SW)).astype(np.float32)

    em = np.asarray(em, dtype=np.float32)
    tg = np.asarray(tg, dtype=np.int64)
    mkb = np.asarray(mk, dtype=bool)
    ar = np.arange(N, dtype=np.int64)[None, :, None]

    in_maps = []
    for c in range(NCORES):
        sl = slice(c * BC, (c + 1) * BC)
        emt = np.ascontiguousarray(
            em[sl].transpose(2, 0, 1)).astype(bf16)
        oht = ((tg[sl][None, :, :] == ar.transpose(1, 0, 2)) &
               mkb[sl][None, :, :]).astype(bf16)
        in_maps.append({
            "emt": emt,
            "oht": np.ascontiguousarray(oht),
            "mk": np.ascontiguousarray(mkb[sl], dtype=np.uint8),
            "tr": np.ascontiguousarray(tr, dtype=np.float32),
            "iota257": iota257,
            "parity": parity,
            "sel2": sel2,
        })

    import os
    res = bass_utils.run_bass_kernel_spmd(
        nc, in_maps, core_ids=list(range(NCORES)), trace=TRACE,
        tmpdir=os.environ.get("BASS_TMPDIR") or None)
    LAST_EXEC_NS = res.exec_time_ns

    parts = [res.results[c]["loss"].reshape(-1) for c in range(NCORES)]
    total = np.concatenate(parts).astype(np.float64)
    return np.float32(total.mean())
